# revision 19
# baseline (speedup 1.0000x reference)
"""Trainium2 Bass kernel for nn_DiffLogicPBF (difflogic network).

Algorithm
---------
The network input is binarized to 2 bits, so every batch row's entire
activation trajectory takes one of only 4 values ("patterns").  We evaluate
the network on the 4 patterns instead of 8192 rows, then blend per-row.

The per-layer gathers (connection indices) are known when the kernel is
built, so they are composed on the host into a stream tree: layer l needs
its layer-(l-1) inputs in 2 permuted orders, giving 2^(5-l) "streams" per
layer (63 total), each a gather-free elementwise evaluation.  Weights are
uploaded pre-permuted per stream; softmax/logic-coefficient math runs on
device via exp + pairwise bf16 folds + strided corner-mask reductions (the
16 soft logic functions' truth tables at the 4 binary corners are exact
bit masks).

Sharding: neurons (K=4096) are split across the 8 cores (512 each).  Each
core computes its partial GroupSum table [4 patterns x 2 classes], blends
the full batch against it ([B,2] partial logits), and the host sums the 8
partial outputs (the blend is linear in the table).

Layout: all eval tensors are q-major ([128, 4 patterns, gates]) so the
per-gate coefficient broadcasts put their stride-0 dim in the middle and
keep the innermost dim contiguous (stride-0 innermost runs ~6x slower on
DVE).  Coefficients are packed [c3,c2,c1,c0] so the eval is 4 fused ops.
"""

from contextlib import ExitStack

import ml_dtypes
import numpy as np

import concourse.bacc as bacc
import concourse.bass as bass
import concourse.mybir as mybir
import concourse.tile as tile
from concourse.bass_utils import run_bass_kernel_spmd

F32 = mybir.dt.float32
BF16 = mybir.dt.bfloat16
ADD = mybir.AluOpType.add
SUB = mybir.AluOpType.subtract
MUL = mybir.AluOpType.mult
X = mybir.AxisListType.X
XY = mybir.AxisListType.XY

N_CORES = 8
B, K, L = 8192, 4096, 6
NS = [32, 16, 8, 4, 2, 1]          # streams per layer
NSTOT = sum(NS)                    # 63
KLOC = K // N_CORES                # 512 neurons per core
J = KLOC // 128                    # 4 free chunks per partition
FO = np.cumsum([0] + NS).tolist()  # stream offsets by layer
BROW = B // 128                    # 64 batch rows per partition

# weight pipeline groups as (start_stream, n_streams): layer 0, then the rest
WG = [(0, 32), (32, 31)]
GRP_OF_LAYER = [0, 1, 1, 1, 1, 1]

_compiled = None


def _build_program():
    nc = bacc.Bacc("TRN2", target_bir_lowering=False, debug=False,
                   num_devices=N_CORES)
    walls = [nc.dram_tensor(f"wall{gi}", [128, n * J * 16], BF16,
                            kind="ExternalInput").ap()
             for gi, (s0_, n) in enumerate(WG)]
    ab0in = nc.dram_tensor("ab0in", [128, 2, 4, NS[0] * J], BF16,
                           kind="ExternalInput").ap()
    xclsin = nc.dram_tensor("xclsin", [128, BROW + 1, 2], F32,
                            kind="ExternalInput").ap()
    out = nc.dram_tensor("out", [B, 2], F32, kind="ExternalOutput").ap()

    EXP = mybir.ActivationFunctionType.Exp
    GT = mybir.AluOpType.is_gt

    with tile.TileContext(nc) as tc:
        with ExitStack() as ctx:
            p = ctx.enter_context(tc.tile_pool(name="p", bufs=1))
            psp = ctx.enter_context(tc.tile_pool(name="ps", bufs=1, space="PSUM"))

            # ---- input DMAs.  The three weight chunks ride three
            # independent DMA paths (scalar HWDGE, sync HWDGE, gpsimd
            # SWDGE) so their transfers run concurrently instead of
            # serializing on one queue (~2.3us each).
            wts = []
            half0 = (WG[0][1] // 2) * J * 16
            for gi, (s0_, n) in enumerate(WG):
                wt = p.tile([128, n * J * 16], BF16, tag=f"wt{gi}")
                if gi == 0:
                    nc.scalar.dma_start(wt[:, 0:half0], walls[0][:, 0:half0])
                    nc.sync.dma_start(wt[:, half0:], walls[0][:, half0:])
                else:
                    nc.gpsimd.dma_start(wt[:], walls[gi][:])
                wts.append(wt)
            xct = p.tile([128, BROW + 1, 2], F32)
            nc.sync.dma_start(xct[:], xclsin[:])
            xt = xct[:, 0:BROW, :]
            ct = xct[:, BROW, :]
            abt = p.tile([128, 2, 4, NS[0] * J], BF16)
            nc.scalar.dma_start(abt[:], ab0in[:])

            # blend prep: per-row one-hot pattern indicators U[b, q] for
            # q = s0 + 2*s1 (fills early DVE idle time).  The final blend
            # is then a single dot with the broadcast 4-entry table.
            s0 = p.tile([128, BROW], F32)
            nc.vector.tensor_scalar(s0[:], xt[:, :, 0], 0.0, None, op0=GT)
            s1 = p.tile([128, BROW], F32)
            nc.vector.tensor_scalar(s1[:], xt[:, :, 1], 0.0, None, op0=GT)
            ns0 = p.tile([128, BROW], F32)
            nc.vector.tensor_scalar(ns0[:], s0[:], -1.0, 1.0, op0=MUL, op1=ADD)
            ns1 = p.tile([128, BROW], F32)
            nc.vector.tensor_scalar(ns1[:], s1[:], -1.0, 1.0, op0=MUL, op1=ADD)
            U = p.tile([128, BROW, 4], BF16)
            nc.vector.tensor_tensor(U[:, :, 0], ns0[:], ns1[:], op=MUL)
            nc.vector.tensor_tensor(U[:, :, 1], s0[:], ns1[:], op=MUL)
            nc.vector.tensor_tensor(U[:, :, 2], ns0[:], s1[:], op=MUL)
            nc.vector.tensor_tensor(U[:, :, 3], s0[:], s1[:], op=MUL)
            ones_m = p.tile([128, 128], BF16)
            nc.vector.memset(ones_m[:], 1.0)
            # warm the PE early (keeps the chain live through the real
            # matmul below)
            wm = psp.tile([1, 1], F32)
            nc.tensor.matmul(wm[:], ones_m[0:1, 0:1], ones_m[0:1, 0:1],
                             start=True, stop=True)
            nc.scalar.copy(ones_m[0:1, 0:1], wm[:])

            # ---- per-group coefficient pipeline ----
            # exp (ACT, bf16 out) -> bf16 pairwise folds at 2x (DVE) ->
            # 5 small strided reduces (DVE) -> recip (DVE custom op) ->
            # subtract algebra + packed C-muls (GpSimd, off critical path).
            Cpk = []                     # per-group packed [c3,c2,c1,c0]
            rw = []
            vt = nc.vector.tensor_tensor
            rd = nc.vector.tensor_reduce
            gt = nc.gpsimd.tensor_tensor
            VT = []
            for gi, (s0_, n) in enumerate(WG):
                nsjg = n * J
                E = p.tile([128, nsjg, 16], BF16, tag=f"E{gi}")
                Ef = E[:].rearrange("p n i -> p (n i)")
                if gi == 0:
                    nc.scalar.activation(Ef[:, 0:half0], wts[0][:, 0:half0],
                                         EXP)
                    nc.scalar.activation(Ef[:, half0:], wts[0][:, half0:],
                                         EXP)
                else:
                    nc.scalar.activation(Ef, wts[gi][:], EXP)

                # A8[j] = e[j] + e[j+8]  (folds corner bit3 away) on DVE;
                # the B4/G4 secondary folds go to GpSimd (idle there).
                A8 = p.tile([128, nsjg, 8], BF16, tag=f"A8{gi}")
                vt(A8[:], E[:, :, 0:8], E[:, :, 8:16], op=ADD)
                # B4[j] = e[j+8] + e[j+12]  (upper half pairs, for V00)
                B4 = p.tile([128, nsjg, 4], BF16, tag=f"B4{gi}")
                gt(B4[:], E[:, :, 8:12], E[:, :, 12:16], op=ADD)
                # G4[j] = A8[j] + A8[j+4]  (for D)
                G4 = p.tile([128, nsjg, 4], BF16, tag=f"G4{gi}")
                gt(G4[:], A8[:, :, 0:4], A8[:, :, 4:8], op=ADD)

                # 4-el sums as two pair-fold TTs (first at 2x) instead of
                # one 1x reduce: ~580ns vs ~812ns each.
                P2 = p.tile([128, 5, nsjg, 2], BF16, tag=f"P2{gi}")
                # V11: odd entries of A8 (strided srcs -> 1x, still wins)
                vt(P2[:, 0], A8[:, :, 1::2][:, :, 0:2],
                   A8[:, :, 1::2][:, :, 2:4], op=ADD)
                # V10: {2,3} + {6,7}
                vt(P2[:, 1], A8[:, :, 2:4], A8[:, :, 6:8], op=ADD)
                # V01: {4,5} + {6,7}
                vt(P2[:, 2], A8[:, :, 4:6], A8[:, :, 6:8], op=ADD)
                # V00: B4 pairs
                vt(P2[:, 3], B4[:, :, 0:2], B4[:, :, 2:4], op=ADD)
                # D: G4 pairs
                vt(P2[:, 4], G4[:, :, 0:2], G4[:, :, 2:4], op=ADD)
                V11 = p.tile([128, nsjg], F32, tag=f"V11{gi}")
                vt(V11[:], P2[:, 0, :, 0], P2[:, 0, :, 1], op=ADD)
                V10 = p.tile([128, nsjg], F32, tag=f"V10{gi}")
                vt(V10[:], P2[:, 1, :, 0], P2[:, 1, :, 1], op=ADD)
                V01 = p.tile([128, nsjg], F32, tag=f"V01{gi}")
                vt(V01[:], P2[:, 2, :, 0], P2[:, 2, :, 1], op=ADD)
                V00 = p.tile([128, nsjg], F32, tag=f"V00{gi}")
                vt(V00[:], P2[:, 3, :, 0], P2[:, 3, :, 1], op=ADD)
                D = p.tile([128, nsjg], F32, tag=f"D{gi}")
                vt(D[:], P2[:, 4, :, 0], P2[:, 4, :, 1], op=ADD)
                r = p.tile([128, nsjg], F32, tag=f"r{gi}")
                nc.vector.reciprocal_approx_fast(r[:], D[:])
                rw.append(r)
                VT.append((V11, V10, V01, V00))

            for gi, (s0_, n) in enumerate(WG):
                nsjg = n * J
                V11, V10, V01, V00 = VT[gi]
                r = rw[gi]
                t1 = p.tile([128, nsjg], F32, tag=f"t1{gi}")
                gt(t1[:], V11[:], V10[:], op=SUB)
                t2 = p.tile([128, nsjg], F32, tag=f"t2{gi}")
                gt(t2[:], V01[:], V00[:], op=SUB)
                t3 = p.tile([128, nsjg], F32, tag=f"t3{gi}")
                gt(t3[:], V10[:], V00[:], op=SUB)
                c3u = p.tile([128, nsjg], F32, tag=f"c3u{gi}")
                gt(c3u[:], t1[:], t2[:], op=SUB)
                C = p.tile([128, 4, nsjg], BF16, tag=f"C{gi}")
                gt(C[:, 0, :], c3u[:], r[:], op=MUL)
                gt(C[:, 1, :], t2[:], r[:], op=MUL)
                gt(C[:, 2, :], t3[:], r[:], op=MUL)
                gt(C[:, 3, :], V00[:], r[:], op=MUL)
                Cpk.append(C)

            # ---- evaluate the stream tree on the 4 patterns (q-major) ----
            # m1|m4 are computed packed: P[:,0] = c3*B + c1, P[:,1] = c2*B
            # + c0, then H = P[:,0]*A + P[:,1].  4 DVE ops per layer.
            def eval_layer(l, A, Bv, cp, Hv, nf):
                P = p.tile([128, 2, 4, nf], BF16, tag=f"P{l}")
                c32 = cp[:, 0:2, :].unsqueeze(2).broadcast_to([128, 2, 4, nf])
                c10 = cp[:, 2:4, :].unsqueeze(2).broadcast_to([128, 2, 4, nf])
                bb = Bv.unsqueeze(1).broadcast_to([128, 2, 4, nf])
                nc.vector.tensor_tensor(P[:], bb, c32, op=MUL)
                nc.vector.tensor_tensor(P[:], P[:], c10, op=ADD)
                nc.vector.tensor_tensor(P[:, 0], P[:, 0], A, op=MUL)
                nc.vector.tensor_tensor(Hv, P[:, 0], P[:, 1], op=ADD)

            Hprev = None
            for l in range(L):
                nf = NS[l] * J
                H = p.tile([128, 4, nf], BF16, tag=f"H{l}")
                if l == 0:
                    A = abt[:, 0, :, :]
                    Bv = abt[:, 1, :, :]
                else:
                    A = Hprev[:, :, 0:nf]
                    Bv = Hprev[:, :, nf:2 * nf]
                g = GRP_OF_LAYER[l]
                off = (FO[l] - WG[g][0]) * J
                cp = Cpk[g][:, :, off:off + nf]
                eval_layer(l, A, Bv, cp, H[:], nf)
                Hprev = H[:]

            # ---- partial GroupSum table, broadcast across partitions ----
            Hred = p.tile([128, 4], BF16)
            with nc.allow_low_precision(reason="4-entry table; 0.4% is fine"):
                nc.vector.tensor_reduce(Hred[:], Hprev, axis=X, op=ADD)
            ps1 = psp.tile([128, 4], F32)
            nc.tensor.matmul(ps1[:], ones_m[:], Hred[:], start=True, stop=True)
            g_t = p.tile([128, 4], BF16)
            nc.scalar.copy(g_t[:], ps1[:])

            # ---- per-row blend: z2[b] = sum_q U[b,q] * table[q] ----
            W = p.tile([128, BROW, 4], BF16)
            gb = g_t[:].unsqueeze(1).broadcast_to([128, BROW, 4])
            nc.vector.tensor_tensor(W[:], U[:], gb, op=MUL)
            z2 = p.tile([128, BROW], F32)
            nc.vector.tensor_reduce(z2[:], W[:], axis=X, op=ADD)

            osb = p.tile([128, BROW, 2], F32)
            nc.vector.tensor_scalar(osb[:, :, 0], z2[:], ct[:, 0:1], None, op0=MUL)
            nc.vector.tensor_scalar(osb[:, :, 1], z2[:], ct[:, 1:2], None, op0=MUL)
            nc.sync.dma_start(out.rearrange("(p a) c -> p a c", p=128), osb[:])

    nc.compile()
    return nc


def _host_blobs(x, w0, ws, idx0, idxs):
    """Compose the stream tree and build per-core input blobs."""
    x = np.asarray(x, np.float32)
    Wl = [np.asarray(w0, np.float32)] + [np.asarray(ws[i], np.float32)
                                         for i in range(L - 1)]
    Il = [np.asarray(idx0, np.int64)] + [np.asarray(idxs[i], np.int64)
                                         for i in range(L - 1)]

    S = [None] * L
    S[L - 1] = [np.arange(K)]
    for l in range(L - 1, 0, -1):
        S[l - 1] = [Il[l][0][P] for P in S[l]] + [Il[l][1][P] for P in S[l]]

    # wall: [cores, 128, (l,s), J, 16]
    wall = np.empty((N_CORES, 128, NSTOT, J, 16), np.float32)
    for l in range(L):
        for s in range(NS[l]):
            pw = Wl[l][S[l][s]]                       # [K, 16]
            pw = pw.reshape(N_CORES, J, 128, 16)      # core, j, p, i
            wall[:, :, FO[l] + s, :, :] = pw.transpose(0, 2, 1, 3)

    # layer-0 pattern inputs: a0[core, p, s, j, q] = (q >> m0) & 1
    q = np.arange(4)
    a0 = np.empty((N_CORES, 128, NS[0], J, 4), np.float32)
    b0 = np.empty((N_CORES, 128, NS[0], J, 4), np.float32)
    for s in range(NS[0]):
        m0 = Il[0][0][S[0][s]].reshape(N_CORES, J, 128)  # core, j, p
        m1 = Il[0][1][S[0][s]].reshape(N_CORES, J, 128)
        a0[:, :, s, :, :] = ((q[None, None, None, :] >> m0.transpose(0, 2, 1)[..., None]) & 1)
        b0[:, :, s, :, :] = ((q[None, None, None, :] >> m1.transpose(0, 2, 1)[..., None]) & 1)
    # q-major: [c, p, q, s*j]
    a0 = a0.transpose(0, 1, 4, 2, 3).reshape(N_CORES, 128, 4, NS[0] * J)
    b0 = b0.transpose(0, 1, 4, 2, 3).reshape(N_CORES, 128, 4, NS[0] * J)

    xin = np.ascontiguousarray(x.reshape(128, BROW, 2))
    in_maps = []
    for ci in range(N_CORES):
        cls = np.array([1.0, 0.0] if ci < N_CORES // 2 else [0.0, 1.0], np.float32)
        xcls = np.concatenate(
            [xin, np.tile(cls, (128, 1))[:, None, :]], axis=1)
        ab0 = np.stack([a0[ci], b0[ci]], axis=1).astype(ml_dtypes.bfloat16)
        m = {
            "ab0in": np.ascontiguousarray(ab0),
            "xclsin": np.ascontiguousarray(xcls),
        }
        for gi, (gs, gn) in enumerate(WG):
            m[f"wall{gi}"] = np.ascontiguousarray(
                wall[ci, :, gs:gs + gn, :, :].reshape(128, -1)).astype(
                    ml_dtypes.bfloat16)
        in_maps.append(m)
    return in_maps


def run(inputs, trace=False, trace_kwargs=None):
    global _compiled
    if _compiled is None:
        _compiled = _build_program()
    nc = _compiled
    in_maps = _host_blobs(inputs["x"], inputs["w0"], inputs["ws"],
                          inputs["idx0"], inputs["idxs"])
    res = run_bass_kernel_spmd(nc, in_maps, core_ids=list(range(N_CORES)),
                               trace=trace, **(trace_kwargs or {}))
    total = np.zeros((B, 2), np.float32)
    for ci in range(N_CORES):
        total += res.results[ci]["out"]
    return total, res


def kernel(x, w0, ws, idx0, idxs):
    out, _ = run({"x": x, "w0": w0, "ws": ws, "idx0": idx0, "idxs": idxs})
    return out


# revision 23
# speedup vs baseline: 1.0459x; 1.0459x over previous
"""Trainium2 Bass kernel for nn_DiffLogicPBF (difflogic network).

Algorithm
---------
The network input is binarized to 2 bits, so every batch row's entire
activation trajectory takes one of only 4 values ("patterns").  We evaluate
the network on the 4 patterns instead of 8192 rows, then blend per-row.

The per-layer gathers (connection indices) are known when the kernel is
built, so they are composed on the host into a stream tree: layer l needs
its layer-(l-1) inputs in 2 permuted orders, giving 2^(5-l) "streams" per
layer (63 total), each a gather-free elementwise evaluation.  Weights are
uploaded pre-permuted per stream; softmax/logic-coefficient math runs on
device via exp + pairwise bf16 folds + strided corner-mask reductions (the
16 soft logic functions' truth tables at the 4 binary corners are exact
bit masks).

Sharding: neurons (K=4096) are split across the 8 cores (512 each).  Each
core computes its partial GroupSum table [4 patterns x 2 classes], blends
the full batch against it ([B,2] partial logits), and the host sums the 8
partial outputs (the blend is linear in the table).

Layout: all eval tensors are q-major ([128, 4 patterns, gates]) so the
per-gate coefficient broadcasts put their stride-0 dim in the middle and
keep the innermost dim contiguous (stride-0 innermost runs ~6x slower on
DVE).  Coefficients are packed [c3,c2,c1,c0] so the eval is 4 fused ops.
"""

from contextlib import ExitStack

import ml_dtypes
import numpy as np

import concourse.bacc as bacc
import concourse.bass as bass
import concourse.mybir as mybir
import concourse.tile as tile
from concourse.bass_utils import run_bass_kernel_spmd

F32 = mybir.dt.float32
BF16 = mybir.dt.bfloat16
ADD = mybir.AluOpType.add
SUB = mybir.AluOpType.subtract
MUL = mybir.AluOpType.mult
X = mybir.AxisListType.X
XY = mybir.AxisListType.XY

N_CORES = 8
B, K, L = 8192, 4096, 6
NS = [32, 16, 8, 4, 2, 1]          # streams per layer
NSTOT = sum(NS)                    # 63
KLOC = K // N_CORES                # 512 neurons per core
J = KLOC // 128                    # 4 free chunks per partition
FO = np.cumsum([0] + NS).tolist()  # stream offsets by layer
BROW = B // 128                    # 64 batch rows per partition

# weight pipeline groups as (start_stream, n_streams): layer 0, then the rest
WG = [(0, 32), (32, 31)]
GRP_OF_LAYER = [0, 1, 1, 1, 1, 1]

_compiled = None


def _build_program():
    nc = bacc.Bacc("TRN2", target_bir_lowering=False, debug=False,
                   num_devices=N_CORES)
    walls = [nc.dram_tensor(f"wall{gi}", [128, n * J * 16], BF16,
                            kind="ExternalInput").ap()
             for gi, (s0_, n) in enumerate(WG)]
    ab0in = nc.dram_tensor("ab0in", [128, 2, 4, NS[0] * J], BF16,
                           kind="ExternalInput").ap()
    xclsin = nc.dram_tensor("xclsin", [128, BROW + 1, 2], F32,
                            kind="ExternalInput").ap()
    out = nc.dram_tensor("out", [B, 2], F32, kind="ExternalOutput").ap()

    EXP = mybir.ActivationFunctionType.Exp
    GT = mybir.AluOpType.is_gt

    with tile.TileContext(nc) as tc:
        with ExitStack() as ctx:
            p = ctx.enter_context(tc.tile_pool(name="p", bufs=1))
            psp = ctx.enter_context(tc.tile_pool(name="ps", bufs=1, space="PSUM"))

            # ---- input DMAs.  The three weight chunks ride three
            # independent DMA paths (scalar HWDGE, sync HWDGE, gpsimd
            # SWDGE) so their transfers run concurrently instead of
            # serializing on one queue (~2.3us each).
            xct = p.tile([128, BROW + 1, 2], F32)
            nc.sync.dma_start(xct[:], xclsin[:])
            xt = xct[:, 0:BROW, :]
            ct = xct[:, BROW, :]
            wts = []
            half0 = (WG[0][1] // 2) * J * 16
            for gi, (s0_, n) in enumerate(WG):
                wt = p.tile([128, n * J * 16], BF16, tag=f"wt{gi}")
                if gi == 0:
                    nc.scalar.dma_start(wt[:, 0:half0], walls[0][:, 0:half0])
                    nc.sync.dma_start(wt[:, half0:], walls[0][:, half0:])
                else:
                    nc.gpsimd.dma_start(wt[:], walls[gi][:])
                wts.append(wt)
            # ab0 (262KB) is only needed by the L0 eval (~10us later than
            # the weights); put it on the SWDGE queue behind wall1 so it
            # doesn't steal HBM bandwidth from the weight transfers.
            abt = p.tile([128, 2, 4, NS[0] * J], BF16)
            nc.gpsimd.dma_start(abt[:], ab0in[:])

            # blend prep: per-row one-hot pattern indicators U[b, q] for
            # q = s0 + 2*s1 (fills early DVE idle time).  The final blend
            # is then a single dot with the broadcast 4-entry table.
            s0 = p.tile([128, BROW], F32)
            nc.vector.tensor_scalar(s0[:], xt[:, :, 0], 0.0, None, op0=GT)
            s1 = p.tile([128, BROW], F32)
            nc.vector.tensor_scalar(s1[:], xt[:, :, 1], 0.0, None, op0=GT)
            ns0 = p.tile([128, BROW], F32)
            nc.vector.tensor_scalar(ns0[:], s0[:], -1.0, 1.0, op0=MUL, op1=ADD)
            ns1 = p.tile([128, BROW], F32)
            nc.vector.tensor_scalar(ns1[:], s1[:], -1.0, 1.0, op0=MUL, op1=ADD)
            U = p.tile([128, BROW, 4], BF16)
            nc.vector.tensor_tensor(U[:, :, 0], ns0[:], ns1[:], op=MUL)
            nc.vector.tensor_tensor(U[:, :, 1], s0[:], ns1[:], op=MUL)
            nc.vector.tensor_tensor(U[:, :, 2], ns0[:], s1[:], op=MUL)
            nc.vector.tensor_tensor(U[:, :, 3], s0[:], s1[:], op=MUL)
            ones_m = p.tile([128, 128], BF16)
            nc.vector.memset(ones_m[:], 1.0)
            # warm the PE early (keeps the chain live through the real
            # matmul below)
            wm = psp.tile([1, 1], F32)
            nc.tensor.matmul(wm[:], ones_m[0:1, 0:1], ones_m[0:1, 0:1],
                             start=True, stop=True)
            nc.scalar.copy(ones_m[0:1, 0:1], wm[:])

            # ---- per-group coefficient pipeline ----
            # exp (ACT, bf16 out) -> bf16 pairwise folds at 2x (DVE) ->
            # 5 small strided reduces (DVE) -> recip (DVE custom op) ->
            # subtract algebra + packed C-muls (GpSimd, off critical path).
            Cpk = []                     # per-group packed [c3,c2,c1,c0]
            rw = []
            vt = nc.vector.tensor_tensor
            rd = nc.vector.tensor_reduce
            gt = nc.gpsimd.tensor_tensor
            VT = []
            folds = []
            for gi, (s0_, n) in enumerate(WG):
                nsjg = n * J
                E = p.tile([128, nsjg, 16], BF16, tag=f"E{gi}")
                Ef = E[:].rearrange("p n i -> p (n i)")
                if gi == 0:
                    nc.scalar.activation(Ef[:, 0:half0], wts[0][:, 0:half0],
                                         EXP)
                    nc.scalar.activation(Ef[:, half0:], wts[0][:, half0:],
                                         EXP)
                else:
                    nc.scalar.activation(Ef, wts[gi][:], EXP)

                # A8[j] = e[j] + e[j+8]  (folds corner bit3 away) on DVE;
                # the B4/G4 secondary folds go to GpSimd (idle there).
                A8 = p.tile([128, nsjg, 8], BF16, tag=f"A8{gi}")
                vt(A8[:], E[:, :, 0:8], E[:, :, 8:16], op=ADD)
                # B4[j] = e[j+8] + e[j+12]  (upper half pairs, for V00)
                B4 = p.tile([128, nsjg, 4], BF16, tag=f"B4{gi}")
                gt(B4[:], E[:, :, 8:12], E[:, :, 12:16], op=ADD)
                # G4[j] = A8[j] + A8[j+4]  (for D)
                G4 = p.tile([128, nsjg, 4], BF16, tag=f"G4{gi}")
                gt(G4[:], A8[:, :, 0:4], A8[:, :, 4:8], op=ADD)
                folds.append((A8, B4, G4))

            for gi, (s0_, n) in enumerate(WG):
                nsjg = n * J
                A8, B4, G4 = folds[gi]
                V11 = p.tile([128, nsjg], F32, tag=f"V11{gi}")
                rd(V11[:], A8[:, :, 1::2], axis=X, op=ADD)
                a0ap = A8[:, :, 0:1]
                V10 = p.tile([128, nsjg], F32, tag=f"V10{gi}")
                m10 = bass.AP(tensor=a0ap.tensor, offset=a0ap.offset + 2,
                              ap=[a0ap.ap[0], [8, nsjg], [4, 2], [1, 2]])
                rd(V10[:], m10, axis=XY, op=ADD)
                V01 = p.tile([128, nsjg], F32, tag=f"V01{gi}")
                rd(V01[:], A8[:, :, 4:8], axis=X, op=ADD)
                V00 = p.tile([128, nsjg], F32, tag=f"V00{gi}")
                rd(V00[:], B4[:], axis=X, op=ADD)
                D = p.tile([128, nsjg], F32, tag=f"D{gi}")
                rd(D[:], G4[:], axis=X, op=ADD)
                r = p.tile([128, nsjg], F32, tag=f"r{gi}")
                nc.vector.reciprocal_approx_fast(r[:], D[:])
                rw.append(r)
                VT.append((V11, V10, V01, V00))

            for gi, (s0_, n) in enumerate(WG):
                nsjg = n * J
                V11, V10, V01, V00 = VT[gi]
                r = rw[gi]
                t1 = p.tile([128, nsjg], F32, tag=f"t1{gi}")
                gt(t1[:], V11[:], V10[:], op=SUB)
                t2 = p.tile([128, nsjg], F32, tag=f"t2{gi}")
                gt(t2[:], V01[:], V00[:], op=SUB)
                t3 = p.tile([128, nsjg], F32, tag=f"t3{gi}")
                gt(t3[:], V10[:], V00[:], op=SUB)
                c3u = p.tile([128, nsjg], F32, tag=f"c3u{gi}")
                gt(c3u[:], t1[:], t2[:], op=SUB)
                C = p.tile([128, 4, nsjg], BF16, tag=f"C{gi}")
                gt(C[:, 0, :], c3u[:], r[:], op=MUL)
                gt(C[:, 1, :], t2[:], r[:], op=MUL)
                gt(C[:, 2, :], t3[:], r[:], op=MUL)
                gt(C[:, 3, :], V00[:], r[:], op=MUL)
                Cpk.append(C)

            # ---- evaluate the stream tree on the 4 patterns (q-major) ----
            # m1|m4 are computed packed: P[:,0] = c3*B + c1, P[:,1] = c2*B
            # + c0, then H = P[:,0]*A + P[:,1].  4 DVE ops per layer.
            def eval_layer(l, A, Bv, cp, Hv, nf):
                P = p.tile([128, 2, 4, nf], BF16, tag=f"P{l}")
                c32 = cp[:, 0:2, :].unsqueeze(2).broadcast_to([128, 2, 4, nf])
                c10 = cp[:, 2:4, :].unsqueeze(2).broadcast_to([128, 2, 4, nf])
                bb = Bv.unsqueeze(1).broadcast_to([128, 2, 4, nf])
                nc.vector.tensor_tensor(P[:], bb, c32, op=MUL)
                nc.vector.tensor_tensor(P[:], P[:], c10, op=ADD)
                nc.vector.tensor_tensor(P[:, 0], P[:, 0], A, op=MUL)
                nc.vector.tensor_tensor(Hv, P[:, 0], P[:, 1], op=ADD)

            Hprev = None
            for l in range(L):
                nf = NS[l] * J
                H = p.tile([128, 4, nf], BF16, tag=f"H{l}")
                if l == 0:
                    A = abt[:, 0, :, :]
                    Bv = abt[:, 1, :, :]
                else:
                    A = Hprev[:, :, 0:nf]
                    Bv = Hprev[:, :, nf:2 * nf]
                g = GRP_OF_LAYER[l]
                off = (FO[l] - WG[g][0]) * J
                cp = Cpk[g][:, :, off:off + nf]
                eval_layer(l, A, Bv, cp, H[:], nf)
                Hprev = H[:]

            # ---- partial GroupSum table, broadcast across partitions ----
            Hred = p.tile([128, 4], BF16)
            with nc.allow_low_precision(reason="4-entry table; 0.4% is fine"):
                nc.vector.tensor_reduce(Hred[:], Hprev, axis=X, op=ADD)
            ps1 = psp.tile([128, 4], F32)
            nc.tensor.matmul(ps1[:], ones_m[:], Hred[:], start=True, stop=True)
            g_t = p.tile([128, 4], BF16)
            nc.scalar.copy(g_t[:], ps1[:])

            # ---- per-row blend: z2[b] = sum_q U[b,q] * table[q] ----
            W = p.tile([128, BROW, 4], BF16)
            gb = g_t[:].unsqueeze(1).broadcast_to([128, BROW, 4])
            nc.vector.tensor_tensor(W[:], U[:], gb, op=MUL)
            z2 = p.tile([128, BROW], F32)
            nc.vector.tensor_reduce(z2[:], W[:], axis=X, op=ADD)

            osb = p.tile([128, BROW, 2], F32)
            nc.vector.tensor_scalar(osb[:, :, 0], z2[:], ct[:, 0:1], None, op0=MUL)
            nc.vector.tensor_scalar(osb[:, :, 1], z2[:], ct[:, 1:2], None, op0=MUL)
            nc.sync.dma_start(out.rearrange("(p a) c -> p a c", p=128), osb[:])

    nc.compile()
    return nc


def _host_blobs(x, w0, ws, idx0, idxs):
    """Compose the stream tree and build per-core input blobs."""
    x = np.asarray(x, np.float32)
    Wl = [np.asarray(w0, np.float32)] + [np.asarray(ws[i], np.float32)
                                         for i in range(L - 1)]
    Il = [np.asarray(idx0, np.int64)] + [np.asarray(idxs[i], np.int64)
                                         for i in range(L - 1)]

    S = [None] * L
    S[L - 1] = [np.arange(K)]
    for l in range(L - 1, 0, -1):
        S[l - 1] = [Il[l][0][P] for P in S[l]] + [Il[l][1][P] for P in S[l]]

    # wall: [cores, 128, (l,s), J, 16]
    wall = np.empty((N_CORES, 128, NSTOT, J, 16), np.float32)
    for l in range(L):
        for s in range(NS[l]):
            pw = Wl[l][S[l][s]]                       # [K, 16]
            pw = pw.reshape(N_CORES, J, 128, 16)      # core, j, p, i
            wall[:, :, FO[l] + s, :, :] = pw.transpose(0, 2, 1, 3)

    # layer-0 pattern inputs: a0[core, p, s, j, q] = (q >> m0) & 1
    q = np.arange(4)
    a0 = np.empty((N_CORES, 128, NS[0], J, 4), np.float32)
    b0 = np.empty((N_CORES, 128, NS[0], J, 4), np.float32)
    for s in range(NS[0]):
        m0 = Il[0][0][S[0][s]].reshape(N_CORES, J, 128)  # core, j, p
        m1 = Il[0][1][S[0][s]].reshape(N_CORES, J, 128)
        a0[:, :, s, :, :] = ((q[None, None, None, :] >> m0.transpose(0, 2, 1)[..., None]) & 1)
        b0[:, :, s, :, :] = ((q[None, None, None, :] >> m1.transpose(0, 2, 1)[..., None]) & 1)
    # q-major: [c, p, q, s*j]
    a0 = a0.transpose(0, 1, 4, 2, 3).reshape(N_CORES, 128, 4, NS[0] * J)
    b0 = b0.transpose(0, 1, 4, 2, 3).reshape(N_CORES, 128, 4, NS[0] * J)

    xin = np.ascontiguousarray(x.reshape(128, BROW, 2))
    in_maps = []
    for ci in range(N_CORES):
        cls = np.array([1.0, 0.0] if ci < N_CORES // 2 else [0.0, 1.0], np.float32)
        xcls = np.concatenate(
            [xin, np.tile(cls, (128, 1))[:, None, :]], axis=1)
        ab0 = np.stack([a0[ci], b0[ci]], axis=1).astype(ml_dtypes.bfloat16)
        m = {
            "ab0in": np.ascontiguousarray(ab0),
            "xclsin": np.ascontiguousarray(xcls),
        }
        for gi, (gs, gn) in enumerate(WG):
            m[f"wall{gi}"] = np.ascontiguousarray(
                wall[ci, :, gs:gs + gn, :, :].reshape(128, -1)).astype(
                    ml_dtypes.bfloat16)
        in_maps.append(m)
    return in_maps


def run(inputs, trace=False, trace_kwargs=None):
    global _compiled
    if _compiled is None:
        _compiled = _build_program()
    nc = _compiled
    in_maps = _host_blobs(inputs["x"], inputs["w0"], inputs["ws"],
                          inputs["idx0"], inputs["idxs"])
    res = run_bass_kernel_spmd(nc, in_maps, core_ids=list(range(N_CORES)),
                               trace=trace, **(trace_kwargs or {}))
    total = np.zeros((B, 2), np.float32)
    for ci in range(N_CORES):
        total += res.results[ci]["out"]
    return total, res


def kernel(x, w0, ws, idx0, idxs):
    out, _ = run({"x": x, "w0": w0, "ws": ws, "idx0": idx0, "idxs": idxs})
    return out


# revision 25
# speedup vs baseline: 1.0656x; 1.0188x over previous
"""Trainium2 Bass kernel for nn_DiffLogicPBF (difflogic network).

Algorithm
---------
The network input is binarized to 2 bits, so every batch row's entire
activation trajectory takes one of only 4 values ("patterns").  We evaluate
the network on the 4 patterns instead of 8192 rows, then blend per-row.

The per-layer gathers (connection indices) are known when the kernel is
built, so they are composed on the host into a stream tree: layer l needs
its layer-(l-1) inputs in 2 permuted orders, giving 2^(5-l) "streams" per
layer (63 total), each a gather-free elementwise evaluation.  Weights are
uploaded pre-permuted per stream; softmax/logic-coefficient math runs on
device via exp + pairwise bf16 folds + strided corner-mask reductions (the
16 soft logic functions' truth tables at the 4 binary corners are exact
bit masks).

Sharding: neurons (K=4096) are split across the 8 cores (512 each).  Each
core computes its partial GroupSum table [4 patterns x 2 classes], blends
the full batch against it ([B,2] partial logits), and the host sums the 8
partial outputs (the blend is linear in the table).

Layout: all eval tensors are q-major ([128, 4 patterns, gates]) so the
per-gate coefficient broadcasts put their stride-0 dim in the middle and
keep the innermost dim contiguous (stride-0 innermost runs ~6x slower on
DVE).  Coefficients are packed [c3,c2,c1,c0] so the eval is 4 fused ops.
"""

from contextlib import ExitStack

import ml_dtypes
import numpy as np

import concourse.bacc as bacc
import concourse.bass as bass
import concourse.mybir as mybir
import concourse.tile as tile
from concourse.bass_utils import run_bass_kernel_spmd

F32 = mybir.dt.float32
BF16 = mybir.dt.bfloat16
ADD = mybir.AluOpType.add
SUB = mybir.AluOpType.subtract
MUL = mybir.AluOpType.mult
X = mybir.AxisListType.X
XY = mybir.AxisListType.XY

N_CORES = 8
B, K, L = 8192, 4096, 6
NS = [32, 16, 8, 4, 2, 1]          # streams per layer
NSTOT = sum(NS)                    # 63
KLOC = K // N_CORES                # 512 neurons per core
J = KLOC // 128                    # 4 free chunks per partition
FO = np.cumsum([0] + NS).tolist()  # stream offsets by layer
BROW = B // 128                    # 64 batch rows per partition

# weight pipeline groups as (start_stream, n_streams): layer 0, then the rest
WG = [(0, 32), (32, 31)]
GRP_OF_LAYER = [0, 1, 1, 1, 1, 1]

_compiled = None


def _build_program():
    nc = bacc.Bacc("TRN2", target_bir_lowering=False, debug=False,
                   num_devices=N_CORES)
    walls = [nc.dram_tensor(f"wall{gi}", [128, n * J * 16], BF16,
                            kind="ExternalInput").ap()
             for gi, (s0_, n) in enumerate(WG)]
    ab0in = nc.dram_tensor("ab0in", [128, 2, 4, NS[0] * J], BF16,
                           kind="ExternalInput").ap()
    xclsin = nc.dram_tensor("xclsin", [128, BROW + 1, 2], F32,
                            kind="ExternalInput").ap()
    out = nc.dram_tensor("out", [B, 2], F32, kind="ExternalOutput").ap()

    EXP = mybir.ActivationFunctionType.Exp
    GT = mybir.AluOpType.is_gt

    with tile.TileContext(nc) as tc:
        with ExitStack() as ctx:
            p = ctx.enter_context(tc.tile_pool(name="p", bufs=1))
            psp = ctx.enter_context(tc.tile_pool(name="ps", bufs=1, space="PSUM"))

            # ---- input DMAs.  The three weight chunks ride three
            # independent DMA paths (scalar HWDGE, sync HWDGE, gpsimd
            # SWDGE) so their transfers run concurrently instead of
            # serializing on one queue (~2.3us each).
            xct = p.tile([128, BROW + 1, 2], F32)
            nc.sync.dma_start(xct[:], xclsin[:])
            xt = xct[:, 0:BROW, :]
            ct = xct[:, BROW, :]
            wts = []
            half0 = (WG[0][1] // 2) * J * 16
            for gi, (s0_, n) in enumerate(WG):
                wt = p.tile([128, n * J * 16], BF16, tag=f"wt{gi}")
                if gi == 0:
                    nc.scalar.dma_start(wt[:, 0:half0], walls[0][:, 0:half0])
                    nc.sync.dma_start(wt[:, half0:], walls[0][:, half0:])
                else:
                    nc.gpsimd.dma_start(wt[:], walls[gi][:])
                wts.append(wt)
            # ab0 (262KB) is only needed by the L0 eval (~10us later than
            # the weights); put it on the SWDGE queue behind wall1 so it
            # doesn't steal HBM bandwidth from the weight transfers.
            abt = p.tile([128, 2, 4, NS[0] * J], BF16)
            nc.gpsimd.dma_start(abt[:], ab0in[:])

            # blend prep: per-row one-hot pattern indicators U[b, q] for
            # q = s0 + 2*s1 (fills early DVE idle time).  The final blend
            # is then a single dot with the broadcast 4-entry table.
            s0 = p.tile([128, BROW], F32)
            nc.vector.tensor_scalar(s0[:], xt[:, :, 0], 0.0, None, op0=GT)
            s1 = p.tile([128, BROW], F32)
            nc.vector.tensor_scalar(s1[:], xt[:, :, 1], 0.0, None, op0=GT)
            ns0 = p.tile([128, BROW], F32)
            nc.vector.tensor_scalar(ns0[:], s0[:], -1.0, 1.0, op0=MUL, op1=ADD)
            ns1 = p.tile([128, BROW], F32)
            nc.vector.tensor_scalar(ns1[:], s1[:], -1.0, 1.0, op0=MUL, op1=ADD)
            U = p.tile([128, BROW, 4], BF16)
            nc.vector.tensor_tensor(U[:, :, 0], ns0[:], ns1[:], op=MUL)
            nc.vector.tensor_tensor(U[:, :, 1], s0[:], ns1[:], op=MUL)
            nc.vector.tensor_tensor(U[:, :, 2], ns0[:], s1[:], op=MUL)
            nc.vector.tensor_tensor(U[:, :, 3], s0[:], s1[:], op=MUL)
            ones_m = p.tile([128, 128], BF16)
            nc.vector.memset(ones_m[:], 1.0)
            # warm the PE early (keeps the chain live through the real
            # matmul below)
            wm = psp.tile([1, 1], F32)
            nc.tensor.matmul(wm[:], ones_m[0:1, 0:1], ones_m[0:1, 0:1],
                             start=True, stop=True)
            nc.scalar.copy(ones_m[0:1, 0:1], wm[:])

            # ---- per-group coefficient pipeline ----
            # exp (ACT, bf16 out) -> bf16 pairwise folds at 2x (DVE) ->
            # 5 small strided reduces (DVE) -> recip (DVE custom op) ->
            # subtract algebra + packed C-muls (GpSimd, off critical path).
            Cpk = []                     # per-group packed [c3,c2,c1,c0]
            rw = []
            vt = nc.vector.tensor_tensor
            rd = nc.vector.tensor_reduce
            gt = nc.gpsimd.tensor_tensor
            VT = []
            folds = []
            for gi, (s0_, n) in enumerate(WG):
                nsjg = n * J
                E = p.tile([128, nsjg, 16], BF16, tag=f"E{gi}")
                Ef = E[:].rearrange("p n i -> p (n i)")
                if gi == 0:
                    nc.scalar.activation(Ef[:, 0:half0], wts[0][:, 0:half0],
                                         EXP)
                    nc.scalar.activation(Ef[:, half0:], wts[0][:, half0:],
                                         EXP)
                else:
                    nc.scalar.activation(Ef, wts[gi][:], EXP)

                # A8[j] = e[j] + e[j+8]  (folds corner bit3 away) on DVE;
                # the B4/G4 secondary folds go to GpSimd (idle there).
                A8 = p.tile([128, nsjg, 8], BF16, tag=f"A8{gi}")
                vt(A8[:], E[:, :, 0:8], E[:, :, 8:16], op=ADD)
                # B4[j] = e[j+8] + e[j+12]  (upper half pairs, for V00)
                B4 = p.tile([128, nsjg, 4], BF16, tag=f"B4{gi}")
                gt(B4[:], E[:, :, 8:12], E[:, :, 12:16], op=ADD)
                # G4[j] = A8[j] + A8[j+4]  (for D)
                G4 = p.tile([128, nsjg, 4], BF16, tag=f"G4{gi}")
                gt(G4[:], A8[:, :, 0:4], A8[:, :, 4:8], op=ADD)
                folds.append((A8, B4, G4))

            for gi, (s0_, n) in enumerate(WG):
                nsjg = n * J
                A8, B4, G4 = folds[gi]
                # all five 4-el sums land in one packed tile so the algebra
                # can work on row pairs: rows = V11, V10, V01, V00, D
                VP = p.tile([128, 5, nsjg], F32, tag=f"VP{gi}")
                rd(VP[:, 0, :], A8[:, :, 1::2], axis=X, op=ADD)
                a0ap = A8[:, :, 0:1]
                m10 = bass.AP(tensor=a0ap.tensor, offset=a0ap.offset + 2,
                              ap=[a0ap.ap[0], [8, nsjg], [4, 2], [1, 2]])
                rd(VP[:, 1, :], m10, axis=XY, op=ADD)
                rd(VP[:, 2, :], A8[:, :, 4:8], axis=X, op=ADD)
                rd(VP[:, 3, :], B4[:], axis=X, op=ADD)
                rd(VP[:, 4, :], G4[:], axis=X, op=ADD)
                r = p.tile([128, nsjg], F32, tag=f"r{gi}")
                nc.vector.reciprocal_approx_fast(r[:], VP[:, 4, :])
                rw.append(r)
                VT.append(VP)

            for gi, (s0_, n) in enumerate(WG):
                nsjg = n * J
                VP = VT[gi]
                r = rw[gi]
                # M rows: [t1->c3u, t2, t3, V00]; one strided GP op computes
                # [t1|t2] = [V11|V01] - [V10|V00], then c3u = t1 - t2 in
                # place, then ONE mul by the broadcast 1/D builds all of C.
                M = p.tile([128, 3, nsjg], F32, tag=f"M{gi}")
                gt(M[:, 0:2, :], VP[:, 0:3:2, :], VP[:, 1:4:2, :], op=SUB)
                gt(M[:, 2, :], VP[:, 1, :], VP[:, 3, :], op=SUB)
                gt(M[:, 0, :], M[:, 0, :], M[:, 1, :], op=SUB)
                C = p.tile([128, 4, nsjg], BF16, tag=f"C{gi}")
                rb = r[:].unsqueeze(1).broadcast_to([128, 3, nsjg])
                gt(C[:, 0:3, :], M[:], rb, op=MUL)
                gt(C[:, 3, :], VP[:, 3, :], r[:], op=MUL)
                Cpk.append(C)

            # ---- evaluate the stream tree on the 4 patterns (q-major) ----
            # m1|m4 are computed packed: P[:,0] = c3*B + c1, P[:,1] = c2*B
            # + c0, then H = P[:,0]*A + P[:,1].  4 DVE ops per layer.
            def eval_layer(l, A, Bv, cp, Hv, nf):
                P = p.tile([128, 2, 4, nf], BF16, tag=f"P{l}")
                c32 = cp[:, 0:2, :].unsqueeze(2).broadcast_to([128, 2, 4, nf])
                c10 = cp[:, 2:4, :].unsqueeze(2).broadcast_to([128, 2, 4, nf])
                bb = Bv.unsqueeze(1).broadcast_to([128, 2, 4, nf])
                nc.vector.tensor_tensor(P[:], bb, c32, op=MUL)
                nc.vector.tensor_tensor(P[:], P[:], c10, op=ADD)
                nc.vector.tensor_tensor(P[:, 0], P[:, 0], A, op=MUL)
                nc.vector.tensor_tensor(Hv, P[:, 0], P[:, 1], op=ADD)

            Hprev = None
            for l in range(L):
                nf = NS[l] * J
                H = p.tile([128, 4, nf], BF16, tag=f"H{l}")
                if l == 0:
                    A = abt[:, 0, :, :]
                    Bv = abt[:, 1, :, :]
                else:
                    A = Hprev[:, :, 0:nf]
                    Bv = Hprev[:, :, nf:2 * nf]
                g = GRP_OF_LAYER[l]
                off = (FO[l] - WG[g][0]) * J
                cp = Cpk[g][:, :, off:off + nf]
                eval_layer(l, A, Bv, cp, H[:], nf)
                Hprev = H[:]

            # ---- partial GroupSum table, broadcast across partitions ----
            Hred = p.tile([128, 4], BF16)
            with nc.allow_low_precision(reason="4-entry table; 0.4% is fine"):
                nc.vector.tensor_reduce(Hred[:], Hprev, axis=X, op=ADD)
            ps1 = psp.tile([128, 4], F32)
            nc.tensor.matmul(ps1[:], ones_m[:], Hred[:], start=True, stop=True)
            g_t = p.tile([128, 4], BF16)
            nc.scalar.copy(g_t[:], ps1[:])

            # ---- per-row blend: z2[b] = sum_q U[b,q] * table[q] ----
            W = p.tile([128, BROW, 4], BF16)
            gb = g_t[:].unsqueeze(1).broadcast_to([128, BROW, 4])
            nc.vector.tensor_tensor(W[:], U[:], gb, op=MUL)
            z2 = p.tile([128, BROW], F32)
            nc.vector.tensor_reduce(z2[:], W[:], axis=X, op=ADD)

            osb = p.tile([128, BROW, 2], F32)
            nc.vector.tensor_scalar(osb[:, :, 0], z2[:], ct[:, 0:1], None, op0=MUL)
            nc.vector.tensor_scalar(osb[:, :, 1], z2[:], ct[:, 1:2], None, op0=MUL)
            nc.sync.dma_start(out.rearrange("(p a) c -> p a c", p=128), osb[:])

    nc.compile()
    return nc


def _host_blobs(x, w0, ws, idx0, idxs):
    """Compose the stream tree and build per-core input blobs."""
    x = np.asarray(x, np.float32)
    Wl = [np.asarray(w0, np.float32)] + [np.asarray(ws[i], np.float32)
                                         for i in range(L - 1)]
    Il = [np.asarray(idx0, np.int64)] + [np.asarray(idxs[i], np.int64)
                                         for i in range(L - 1)]

    S = [None] * L
    S[L - 1] = [np.arange(K)]
    for l in range(L - 1, 0, -1):
        S[l - 1] = [Il[l][0][P] for P in S[l]] + [Il[l][1][P] for P in S[l]]

    # wall: [cores, 128, (l,s), J, 16]
    wall = np.empty((N_CORES, 128, NSTOT, J, 16), np.float32)
    for l in range(L):
        for s in range(NS[l]):
            pw = Wl[l][S[l][s]]                       # [K, 16]
            pw = pw.reshape(N_CORES, J, 128, 16)      # core, j, p, i
            wall[:, :, FO[l] + s, :, :] = pw.transpose(0, 2, 1, 3)

    # layer-0 pattern inputs: a0[core, p, s, j, q] = (q >> m0) & 1
    q = np.arange(4)
    a0 = np.empty((N_CORES, 128, NS[0], J, 4), np.float32)
    b0 = np.empty((N_CORES, 128, NS[0], J, 4), np.float32)
    for s in range(NS[0]):
        m0 = Il[0][0][S[0][s]].reshape(N_CORES, J, 128)  # core, j, p
        m1 = Il[0][1][S[0][s]].reshape(N_CORES, J, 128)
        a0[:, :, s, :, :] = ((q[None, None, None, :] >> m0.transpose(0, 2, 1)[..., None]) & 1)
        b0[:, :, s, :, :] = ((q[None, None, None, :] >> m1.transpose(0, 2, 1)[..., None]) & 1)
    # q-major: [c, p, q, s*j]
    a0 = a0.transpose(0, 1, 4, 2, 3).reshape(N_CORES, 128, 4, NS[0] * J)
    b0 = b0.transpose(0, 1, 4, 2, 3).reshape(N_CORES, 128, 4, NS[0] * J)

    xin = np.ascontiguousarray(x.reshape(128, BROW, 2))
    in_maps = []
    for ci in range(N_CORES):
        cls = np.array([1.0, 0.0] if ci < N_CORES // 2 else [0.0, 1.0], np.float32)
        xcls = np.concatenate(
            [xin, np.tile(cls, (128, 1))[:, None, :]], axis=1)
        ab0 = np.stack([a0[ci], b0[ci]], axis=1).astype(ml_dtypes.bfloat16)
        m = {
            "ab0in": np.ascontiguousarray(ab0),
            "xclsin": np.ascontiguousarray(xcls),
        }
        for gi, (gs, gn) in enumerate(WG):
            m[f"wall{gi}"] = np.ascontiguousarray(
                wall[ci, :, gs:gs + gn, :, :].reshape(128, -1)).astype(
                    ml_dtypes.bfloat16)
        in_maps.append(m)
    return in_maps


def run(inputs, trace=False, trace_kwargs=None):
    global _compiled
    if _compiled is None:
        _compiled = _build_program()
    nc = _compiled
    in_maps = _host_blobs(inputs["x"], inputs["w0"], inputs["ws"],
                          inputs["idx0"], inputs["idxs"])
    res = run_bass_kernel_spmd(nc, in_maps, core_ids=list(range(N_CORES)),
                               trace=trace, **(trace_kwargs or {}))
    total = np.zeros((B, 2), np.float32)
    for ci in range(N_CORES):
        total += res.results[ci]["out"]
    return total, res


def kernel(x, w0, ws, idx0, idxs):
    out, _ = run({"x": x, "w0": w0, "ws": ws, "idx0": idx0, "idxs": idxs})
    return out


# revision 26
# speedup vs baseline: 1.1144x; 1.0457x over previous
"""Trainium2 Bass kernel for nn_DiffLogicPBF (difflogic network).

Algorithm
---------
The network input is binarized to 2 bits, so every batch row's entire
activation trajectory takes one of only 4 values ("patterns").  We evaluate
the network on the 4 patterns instead of 8192 rows, then blend per-row.

The per-layer gathers (connection indices) are known when the kernel is
built, so they are composed on the host into a stream tree: layer l needs
its layer-(l-1) inputs in 2 permuted orders, giving 2^(5-l) "streams" per
layer (63 total), each a gather-free elementwise evaluation.  Weights are
uploaded pre-permuted per stream; softmax/logic-coefficient math runs on
device via exp + pairwise bf16 folds + strided corner-mask reductions (the
16 soft logic functions' truth tables at the 4 binary corners are exact
bit masks).

Sharding: neurons (K=4096) are split across the 8 cores (512 each).  Each
core computes its partial GroupSum table [4 patterns x 2 classes], blends
the full batch against it ([B,2] partial logits), and the host sums the 8
partial outputs (the blend is linear in the table).

Layout: all eval tensors are q-major ([128, 4 patterns, gates]) so the
per-gate coefficient broadcasts put their stride-0 dim in the middle and
keep the innermost dim contiguous (stride-0 innermost runs ~6x slower on
DVE).  Coefficients are packed [c3,c2,c1,c0] so the eval is 4 fused ops.
"""

from contextlib import ExitStack

import ml_dtypes
import numpy as np

import concourse.bacc as bacc
import concourse.bass as bass
import concourse.mybir as mybir
import concourse.tile as tile
from concourse.bass_utils import run_bass_kernel_spmd

F32 = mybir.dt.float32
BF16 = mybir.dt.bfloat16
ADD = mybir.AluOpType.add
SUB = mybir.AluOpType.subtract
MUL = mybir.AluOpType.mult
X = mybir.AxisListType.X
XY = mybir.AxisListType.XY

N_CORES = 8
B, K, L = 8192, 4096, 6
NS = [32, 16, 8, 4, 2, 1]          # streams per layer
NSTOT = sum(NS)                    # 63
KLOC = K // N_CORES                # 512 neurons per core
J = KLOC // 128                    # 4 free chunks per partition
FO = np.cumsum([0] + NS).tolist()  # stream offsets by layer
BROW = B // 128                    # 64 batch rows per partition

# weight pipeline groups as (start_stream, n_streams): layer 0, then the rest
WG = [(0, 32), (32, 31)]
GRP_OF_LAYER = [0, 1, 1, 1, 1, 1]

_compiled = None


def _build_program():
    nc = bacc.Bacc("TRN2", target_bir_lowering=False, debug=False,
                   num_devices=N_CORES)
    F8 = mybir.dt.float8e3
    walls = [nc.dram_tensor(f"wall{gi}", [128, n * J * 16], F8,
                            kind="ExternalInput").ap()
             for gi, (s0_, n) in enumerate(WG)]
    ab0in = nc.dram_tensor("ab0in", [128, 2, 4, NS[0] * J], BF16,
                           kind="ExternalInput").ap()
    xclsin = nc.dram_tensor("xclsin", [128, BROW + 1, 2], F32,
                            kind="ExternalInput").ap()
    out = nc.dram_tensor("out", [B, 2], F32, kind="ExternalOutput").ap()

    EXP = mybir.ActivationFunctionType.Exp
    GT = mybir.AluOpType.is_gt

    with tile.TileContext(nc) as tc:
        with ExitStack() as ctx:
            p = ctx.enter_context(tc.tile_pool(name="p", bufs=1))
            psp = ctx.enter_context(tc.tile_pool(name="ps", bufs=1, space="PSUM"))

            # ---- input DMAs.  The three weight chunks ride three
            # independent DMA paths (scalar HWDGE, sync HWDGE, gpsimd
            # SWDGE) so their transfers run concurrently instead of
            # serializing on one queue (~2.3us each).
            xct = p.tile([128, BROW + 1, 2], F32)
            nc.sync.dma_start(xct[:], xclsin[:])
            xt = xct[:, 0:BROW, :]
            ct = xct[:, BROW, :]
            wts = []
            half0 = (WG[0][1] // 2) * J * 16
            for gi, (s0_, n) in enumerate(WG):
                wt = p.tile([128, n * J * 16], mybir.dt.float8e3, tag=f"wt{gi}")
                if gi == 0:
                    nc.scalar.dma_start(wt[:, 0:half0], walls[0][:, 0:half0])
                    nc.sync.dma_start(wt[:, half0:], walls[0][:, half0:])
                else:
                    nc.gpsimd.dma_start(wt[:], walls[gi][:])
                wts.append(wt)
            # ab0 (262KB) is only needed by the L0 eval (~10us later than
            # the weights); put it on the SWDGE queue behind wall1 so it
            # doesn't steal HBM bandwidth from the weight transfers.
            abt = p.tile([128, 2, 4, NS[0] * J], BF16)
            nc.gpsimd.dma_start(abt[:], ab0in[:])

            # blend prep: per-row one-hot pattern indicators U[b, q] for
            # q = s0 + 2*s1 (fills early DVE idle time).  The final blend
            # is then a single dot with the broadcast 4-entry table.
            s0 = p.tile([128, BROW], F32)
            nc.vector.tensor_scalar(s0[:], xt[:, :, 0], 0.0, None, op0=GT)
            s1 = p.tile([128, BROW], F32)
            nc.vector.tensor_scalar(s1[:], xt[:, :, 1], 0.0, None, op0=GT)
            ns0 = p.tile([128, BROW], F32)
            nc.vector.tensor_scalar(ns0[:], s0[:], -1.0, 1.0, op0=MUL, op1=ADD)
            ns1 = p.tile([128, BROW], F32)
            nc.vector.tensor_scalar(ns1[:], s1[:], -1.0, 1.0, op0=MUL, op1=ADD)
            U = p.tile([128, BROW, 4], BF16)
            nc.vector.tensor_tensor(U[:, :, 0], ns0[:], ns1[:], op=MUL)
            nc.vector.tensor_tensor(U[:, :, 1], s0[:], ns1[:], op=MUL)
            nc.vector.tensor_tensor(U[:, :, 2], ns0[:], s1[:], op=MUL)
            nc.vector.tensor_tensor(U[:, :, 3], s0[:], s1[:], op=MUL)
            ones_m = p.tile([128, 128], BF16)
            nc.vector.memset(ones_m[:], 1.0)
            # warm the PE early (keeps the chain live through the real
            # matmul below)
            wm = psp.tile([1, 1], F32)
            nc.tensor.matmul(wm[:], ones_m[0:1, 0:1], ones_m[0:1, 0:1],
                             start=True, stop=True)
            nc.scalar.copy(ones_m[0:1, 0:1], wm[:])

            # ---- per-group coefficient pipeline ----
            # exp (ACT, bf16 out) -> bf16 pairwise folds at 2x (DVE) ->
            # 5 small strided reduces (DVE) -> recip (DVE custom op) ->
            # subtract algebra + packed C-muls (GpSimd, off critical path).
            Cpk = []                     # per-group packed [c3,c2,c1,c0]
            rw = []
            vt = nc.vector.tensor_tensor
            rd = nc.vector.tensor_reduce
            gt = nc.gpsimd.tensor_tensor
            VT = []
            folds = []
            for gi, (s0_, n) in enumerate(WG):
                nsjg = n * J
                E = p.tile([128, nsjg, 16], BF16, tag=f"E{gi}")
                Ef = E[:].rearrange("p n i -> p (n i)")
                if gi == 0:
                    nc.scalar.activation(Ef[:, 0:half0], wts[0][:, 0:half0],
                                         EXP)
                    nc.scalar.activation(Ef[:, half0:], wts[0][:, half0:],
                                         EXP)
                else:
                    nc.scalar.activation(Ef, wts[gi][:], EXP)

                # A8[j] = e[j] + e[j+8]  (folds corner bit3 away) on DVE;
                # the B4/G4 secondary folds go to GpSimd (idle there).
                A8 = p.tile([128, nsjg, 8], BF16, tag=f"A8{gi}")
                vt(A8[:], E[:, :, 0:8], E[:, :, 8:16], op=ADD)
                # B4[j] = e[j+8] + e[j+12]  (upper half pairs, for V00)
                B4 = p.tile([128, nsjg, 4], BF16, tag=f"B4{gi}")
                gt(B4[:], E[:, :, 8:12], E[:, :, 12:16], op=ADD)
                # G4[j] = A8[j] + A8[j+4]  (for D)
                G4 = p.tile([128, nsjg, 4], BF16, tag=f"G4{gi}")
                gt(G4[:], A8[:, :, 0:4], A8[:, :, 4:8], op=ADD)
                folds.append((A8, B4, G4))

            for gi, (s0_, n) in enumerate(WG):
                nsjg = n * J
                A8, B4, G4 = folds[gi]
                # all five 4-el sums land in one packed tile so the algebra
                # can work on row pairs: rows = V11, V10, V01, V00, D
                VP = p.tile([128, 5, nsjg], F32, tag=f"VP{gi}")
                rd(VP[:, 0, :], A8[:, :, 1::2], axis=X, op=ADD)
                a0ap = A8[:, :, 0:1]
                m10 = bass.AP(tensor=a0ap.tensor, offset=a0ap.offset + 2,
                              ap=[a0ap.ap[0], [8, nsjg], [4, 2], [1, 2]])
                rd(VP[:, 1, :], m10, axis=XY, op=ADD)
                rd(VP[:, 2, :], A8[:, :, 4:8], axis=X, op=ADD)
                rd(VP[:, 3, :], B4[:], axis=X, op=ADD)
                rd(VP[:, 4, :], G4[:], axis=X, op=ADD)
                r = p.tile([128, nsjg], F32, tag=f"r{gi}")
                nc.vector.reciprocal_approx_fast(r[:], VP[:, 4, :])
                rw.append(r)
                VT.append(VP)

            for gi, (s0_, n) in enumerate(WG):
                nsjg = n * J
                VP = VT[gi]
                r = rw[gi]
                # M rows: [t1->c3u, t2, t3, V00]; one strided GP op computes
                # [t1|t2] = [V11|V01] - [V10|V00], then c3u = t1 - t2 in
                # place, then ONE mul by the broadcast 1/D builds all of C.
                M = p.tile([128, 3, nsjg], F32, tag=f"M{gi}")
                gt(M[:, 0:2, :], VP[:, 0:3:2, :], VP[:, 1:4:2, :], op=SUB)
                gt(M[:, 2, :], VP[:, 1, :], VP[:, 3, :], op=SUB)
                gt(M[:, 0, :], M[:, 0, :], M[:, 1, :], op=SUB)
                C = p.tile([128, 4, nsjg], BF16, tag=f"C{gi}")
                rb = r[:].unsqueeze(1).broadcast_to([128, 3, nsjg])
                gt(C[:, 0:3, :], M[:], rb, op=MUL)
                gt(C[:, 3, :], VP[:, 3, :], r[:], op=MUL)
                Cpk.append(C)

            # ---- evaluate the stream tree on the 4 patterns (q-major) ----
            # m1|m4 are computed packed: P[:,0] = c3*B + c1, P[:,1] = c2*B
            # + c0, then H = P[:,0]*A + P[:,1].  4 DVE ops per layer.
            def eval_layer(l, A, Bv, cp, Hv, nf):
                P = p.tile([128, 2, 4, nf], BF16, tag=f"P{l}")
                c32 = cp[:, 0:2, :].unsqueeze(2).broadcast_to([128, 2, 4, nf])
                c10 = cp[:, 2:4, :].unsqueeze(2).broadcast_to([128, 2, 4, nf])
                bb = Bv.unsqueeze(1).broadcast_to([128, 2, 4, nf])
                nc.vector.tensor_tensor(P[:], bb, c32, op=MUL)
                nc.vector.tensor_tensor(P[:], P[:], c10, op=ADD)
                nc.vector.tensor_tensor(P[:, 0], P[:, 0], A, op=MUL)
                nc.vector.tensor_tensor(Hv, P[:, 0], P[:, 1], op=ADD)

            Hprev = None
            for l in range(L):
                nf = NS[l] * J
                H = p.tile([128, 4, nf], BF16, tag=f"H{l}")
                if l == 0:
                    A = abt[:, 0, :, :]
                    Bv = abt[:, 1, :, :]
                else:
                    A = Hprev[:, :, 0:nf]
                    Bv = Hprev[:, :, nf:2 * nf]
                g = GRP_OF_LAYER[l]
                off = (FO[l] - WG[g][0]) * J
                cp = Cpk[g][:, :, off:off + nf]
                eval_layer(l, A, Bv, cp, H[:], nf)
                Hprev = H[:]

            # ---- partial GroupSum table, broadcast across partitions ----
            Hred = p.tile([128, 4], BF16)
            with nc.allow_low_precision(reason="4-entry table; 0.4% is fine"):
                nc.vector.tensor_reduce(Hred[:], Hprev, axis=X, op=ADD)
            ps1 = psp.tile([128, 4], F32)
            nc.tensor.matmul(ps1[:], ones_m[:], Hred[:], start=True, stop=True)
            g_t = p.tile([128, 4], BF16)
            nc.scalar.copy(g_t[:], ps1[:])

            # ---- per-row blend: z2[b] = sum_q U[b,q] * table[q] ----
            W = p.tile([128, BROW, 4], BF16)
            gb = g_t[:].unsqueeze(1).broadcast_to([128, BROW, 4])
            nc.vector.tensor_tensor(W[:], U[:], gb, op=MUL)
            z2 = p.tile([128, BROW], F32)
            nc.vector.tensor_reduce(z2[:], W[:], axis=X, op=ADD)

            osb = p.tile([128, BROW, 2], F32)
            nc.vector.tensor_scalar(osb[:, :, 0], z2[:], ct[:, 0:1], None, op0=MUL)
            nc.vector.tensor_scalar(osb[:, :, 1], z2[:], ct[:, 1:2], None, op0=MUL)
            nc.sync.dma_start(out.rearrange("(p a) c -> p a c", p=128), osb[:])

    nc.compile()
    return nc


def _host_blobs(x, w0, ws, idx0, idxs):
    """Compose the stream tree and build per-core input blobs."""
    x = np.asarray(x, np.float32)
    Wl = [np.asarray(w0, np.float32)] + [np.asarray(ws[i], np.float32)
                                         for i in range(L - 1)]
    Il = [np.asarray(idx0, np.int64)] + [np.asarray(idxs[i], np.int64)
                                         for i in range(L - 1)]

    S = [None] * L
    S[L - 1] = [np.arange(K)]
    for l in range(L - 1, 0, -1):
        S[l - 1] = [Il[l][0][P] for P in S[l]] + [Il[l][1][P] for P in S[l]]

    # wall: [cores, 128, (l,s), J, 16]
    wall = np.empty((N_CORES, 128, NSTOT, J, 16), np.float32)
    for l in range(L):
        for s in range(NS[l]):
            pw = Wl[l][S[l][s]]                       # [K, 16]
            pw = pw.reshape(N_CORES, J, 128, 16)      # core, j, p, i
            wall[:, :, FO[l] + s, :, :] = pw.transpose(0, 2, 1, 3)

    # layer-0 pattern inputs: a0[core, p, s, j, q] = (q >> m0) & 1
    q = np.arange(4)
    a0 = np.empty((N_CORES, 128, NS[0], J, 4), np.float32)
    b0 = np.empty((N_CORES, 128, NS[0], J, 4), np.float32)
    for s in range(NS[0]):
        m0 = Il[0][0][S[0][s]].reshape(N_CORES, J, 128)  # core, j, p
        m1 = Il[0][1][S[0][s]].reshape(N_CORES, J, 128)
        a0[:, :, s, :, :] = ((q[None, None, None, :] >> m0.transpose(0, 2, 1)[..., None]) & 1)
        b0[:, :, s, :, :] = ((q[None, None, None, :] >> m1.transpose(0, 2, 1)[..., None]) & 1)
    # q-major: [c, p, q, s*j]
    a0 = a0.transpose(0, 1, 4, 2, 3).reshape(N_CORES, 128, 4, NS[0] * J)
    b0 = b0.transpose(0, 1, 4, 2, 3).reshape(N_CORES, 128, 4, NS[0] * J)

    xin = np.ascontiguousarray(x.reshape(128, BROW, 2))
    in_maps = []
    for ci in range(N_CORES):
        cls = np.array([1.0, 0.0] if ci < N_CORES // 2 else [0.0, 1.0], np.float32)
        xcls = np.concatenate(
            [xin, np.tile(cls, (128, 1))[:, None, :]], axis=1)
        ab0 = np.stack([a0[ci], b0[ci]], axis=1).astype(ml_dtypes.bfloat16)
        m = {
            "ab0in": np.ascontiguousarray(ab0),
            "xclsin": np.ascontiguousarray(xcls),
        }
        for gi, (gs, gn) in enumerate(WG):
            m[f"wall{gi}"] = np.ascontiguousarray(
                wall[ci, :, gs:gs + gn, :, :].reshape(128, -1)).astype(
                    ml_dtypes.float8_e3m4)
        in_maps.append(m)
    return in_maps


def run(inputs, trace=False, trace_kwargs=None):
    global _compiled
    if _compiled is None:
        _compiled = _build_program()
    nc = _compiled
    in_maps = _host_blobs(inputs["x"], inputs["w0"], inputs["ws"],
                          inputs["idx0"], inputs["idxs"])
    res = run_bass_kernel_spmd(nc, in_maps, core_ids=list(range(N_CORES)),
                               trace=trace, **(trace_kwargs or {}))
    total = np.zeros((B, 2), np.float32)
    for ci in range(N_CORES):
        total += res.results[ci]["out"]
    return total, res


def kernel(x, w0, ws, idx0, idxs):
    out, _ = run({"x": x, "w0": w0, "ws": ws, "idx0": idx0, "idxs": idxs})
    return out


# revision 27
# speedup vs baseline: 1.1287x; 1.0129x over previous
"""Trainium2 Bass kernel for nn_DiffLogicPBF (difflogic network).

Algorithm
---------
The network input is binarized to 2 bits, so every batch row's entire
activation trajectory takes one of only 4 values ("patterns").  We evaluate
the network on the 4 patterns instead of 8192 rows, then blend per-row.

The per-layer gathers (connection indices) are known when the kernel is
built, so they are composed on the host into a stream tree: layer l needs
its layer-(l-1) inputs in 2 permuted orders, giving 2^(5-l) "streams" per
layer (63 total), each a gather-free elementwise evaluation.  Weights are
uploaded pre-permuted per stream; softmax/logic-coefficient math runs on
device via exp + pairwise bf16 folds + strided corner-mask reductions (the
16 soft logic functions' truth tables at the 4 binary corners are exact
bit masks).

Sharding: neurons (K=4096) are split across the 8 cores (512 each).  Each
core computes its partial GroupSum table [4 patterns x 2 classes], blends
the full batch against it ([B,2] partial logits), and the host sums the 8
partial outputs (the blend is linear in the table).

Layout: all eval tensors are q-major ([128, 4 patterns, gates]) so the
per-gate coefficient broadcasts put their stride-0 dim in the middle and
keep the innermost dim contiguous (stride-0 innermost runs ~6x slower on
DVE).  Coefficients are packed [c3,c2,c1,c0] so the eval is 4 fused ops.
"""

from contextlib import ExitStack

import ml_dtypes
import numpy as np

import concourse.bacc as bacc
import concourse.bass as bass
import concourse.mybir as mybir
import concourse.tile as tile
from concourse.bass_utils import run_bass_kernel_spmd

F32 = mybir.dt.float32
BF16 = mybir.dt.bfloat16
ADD = mybir.AluOpType.add
SUB = mybir.AluOpType.subtract
MUL = mybir.AluOpType.mult
X = mybir.AxisListType.X
XY = mybir.AxisListType.XY

N_CORES = 8
B, K, L = 8192, 4096, 6
NS = [32, 16, 8, 4, 2, 1]          # streams per layer
NSTOT = sum(NS)                    # 63
KLOC = K // N_CORES                # 512 neurons per core
J = KLOC // 128                    # 4 free chunks per partition
FO = np.cumsum([0] + NS).tolist()  # stream offsets by layer
BROW = B // 128                    # 64 batch rows per partition

# weight pipeline groups as (start_stream, n_streams): layer 0, then the rest
WG = [(0, 32), (32, 31)]
GRP_OF_LAYER = [0, 1, 1, 1, 1, 1]

_compiled = None


def _build_program():
    nc = bacc.Bacc("TRN2", target_bir_lowering=False, debug=False,
                   num_devices=N_CORES)
    F8 = mybir.dt.float8e3
    walls = [nc.dram_tensor(f"wall{gi}", [128, n * J * 16], F8,
                            kind="ExternalInput").ap()
             for gi, (s0_, n) in enumerate(WG)]
    ab0in = nc.dram_tensor("ab0in", [128, 2, 4, NS[0] * J], BF16,
                           kind="ExternalInput").ap()
    xclsin = nc.dram_tensor("xclsin", [128, BROW + 1, 2], F32,
                            kind="ExternalInput").ap()
    out = nc.dram_tensor("out", [B, 2], F32, kind="ExternalOutput").ap()

    EXP = mybir.ActivationFunctionType.Exp
    GT = mybir.AluOpType.is_gt

    with tile.TileContext(nc) as tc:
        with ExitStack() as ctx:
            p = ctx.enter_context(tc.tile_pool(name="p", bufs=1))
            psp = ctx.enter_context(tc.tile_pool(name="ps", bufs=1, space="PSUM"))

            # ---- input DMAs.  The three weight chunks ride three
            # independent DMA paths (scalar HWDGE, sync HWDGE, gpsimd
            # SWDGE) so their transfers run concurrently instead of
            # serializing on one queue (~2.3us each).
            xct = p.tile([128, BROW + 1, 2], F32)
            nc.sync.dma_start(xct[:], xclsin[:])
            xt = xct[:, 0:BROW, :]
            ct = xct[:, BROW, :]
            wts = []
            half0 = (WG[0][1] // 2) * J * 16
            for gi, (s0_, n) in enumerate(WG):
                wt = p.tile([128, n * J * 16], mybir.dt.float8e3, tag=f"wt{gi}")
                if gi == 0:
                    nc.scalar.dma_start(wt[:, 0:half0], walls[0][:, 0:half0])
                    nc.sync.dma_start(wt[:, half0:], walls[0][:, half0:])
                else:
                    nc.gpsimd.dma_start(wt[:], walls[gi][:])
                wts.append(wt)
            # ab0 (262KB) is only needed by the L0 eval (~10us later than
            # the weights); put it on the SWDGE queue behind wall1 so it
            # doesn't steal HBM bandwidth from the weight transfers.
            abt = p.tile([128, 2, 4, NS[0] * J], BF16)
            nc.gpsimd.dma_start(abt[:], ab0in[:])

            # blend prep: per-row one-hot pattern indicators U[b, q] for
            # q = s0 + 2*s1 (fills early DVE idle time).  The final blend
            # is then a single dot with the broadcast 4-entry table.
            s0 = p.tile([128, BROW], F32)
            nc.vector.tensor_scalar(s0[:], xt[:, :, 0], 0.0, None, op0=GT)
            s1 = p.tile([128, BROW], F32)
            nc.vector.tensor_scalar(s1[:], xt[:, :, 1], 0.0, None, op0=GT)
            ns0 = p.tile([128, BROW], F32)
            nc.vector.tensor_scalar(ns0[:], s0[:], -1.0, 1.0, op0=MUL, op1=ADD)
            ns1 = p.tile([128, BROW], F32)
            nc.vector.tensor_scalar(ns1[:], s1[:], -1.0, 1.0, op0=MUL, op1=ADD)
            U = p.tile([128, BROW, 4], BF16)
            nc.vector.tensor_tensor(U[:, :, 0], ns0[:], ns1[:], op=MUL)
            nc.vector.tensor_tensor(U[:, :, 1], s0[:], ns1[:], op=MUL)
            nc.vector.tensor_tensor(U[:, :, 2], ns0[:], s1[:], op=MUL)
            nc.vector.tensor_tensor(U[:, :, 3], s0[:], s1[:], op=MUL)
            ones_m = p.tile([128, 128], BF16)
            nc.vector.memset(ones_m[:], 1.0)
            # warm the PE early (keeps the chain live through the real
            # matmul below)
            wm = psp.tile([1, 1], F32)
            nc.tensor.matmul(wm[:], ones_m[0:1, 0:1], ones_m[0:1, 0:1],
                             start=True, stop=True)
            nc.scalar.copy(ones_m[0:1, 0:1], wm[:])

            # ---- per-group coefficient pipeline ----
            # exp (ACT, bf16 out) -> bf16 pairwise folds at 2x (DVE) ->
            # 5 small strided reduces (DVE) -> recip (DVE custom op) ->
            # subtract algebra + packed C-muls (GpSimd, off critical path).
            Cpk = []                     # per-group packed [c3,c2,c1,c0]
            rw = []
            vt = nc.vector.tensor_tensor
            rd = nc.vector.tensor_reduce
            gt = nc.gpsimd.tensor_tensor
            VT = []
            folds = []
            for gi, (s0_, n) in enumerate(WG):
                nsjg = n * J
                E = p.tile([128, nsjg, 16], BF16, tag=f"E{gi}")
                Ef = E[:].rearrange("p n i -> p (n i)")
                if gi == 0:
                    nc.scalar.activation(Ef[:, 0:half0], wts[0][:, 0:half0],
                                         EXP)
                    nc.scalar.activation(Ef[:, half0:], wts[0][:, half0:],
                                         EXP)
                else:
                    nc.scalar.activation(Ef, wts[gi][:], EXP)

                # A8[j] = e[j] + e[j+8]  (folds corner bit3 away) on DVE;
                # the B4/G4 secondary folds go to GpSimd (idle there).
                # Group 0's fold runs in halves so it starts right after
                # the first exp instead of waiting for both.
                A8 = p.tile([128, nsjg, 8], BF16, tag=f"A8{gi}")
                if gi == 0:
                    h = nsjg // 2
                    vt(A8[:, 0:h, :], E[:, 0:h, 0:8], E[:, 0:h, 8:16],
                       op=ADD)
                    vt(A8[:, h:, :], E[:, h:, 0:8], E[:, h:, 8:16], op=ADD)
                else:
                    vt(A8[:], E[:, :, 0:8], E[:, :, 8:16], op=ADD)
                # B4[j] = e[j+8] + e[j+12]  (upper half pairs, for V00)
                B4 = p.tile([128, nsjg, 4], BF16, tag=f"B4{gi}")
                gt(B4[:], E[:, :, 8:12], E[:, :, 12:16], op=ADD)
                # G4[j] = A8[j] + A8[j+4]  (for D)
                G4 = p.tile([128, nsjg, 4], BF16, tag=f"G4{gi}")
                gt(G4[:], A8[:, :, 0:4], A8[:, :, 4:8], op=ADD)
                folds.append((A8, B4, G4))

            for gi, (s0_, n) in enumerate(WG):
                nsjg = n * J
                A8, B4, G4 = folds[gi]
                # all five 4-el sums land in one packed tile so the algebra
                # can work on row pairs: rows = V11, V10, V01, V00, D
                VP = p.tile([128, 5, nsjg], F32, tag=f"VP{gi}")
                rd(VP[:, 0, :], A8[:, :, 1::2], axis=X, op=ADD)
                a0ap = A8[:, :, 0:1]
                m10 = bass.AP(tensor=a0ap.tensor, offset=a0ap.offset + 2,
                              ap=[a0ap.ap[0], [8, nsjg], [4, 2], [1, 2]])
                rd(VP[:, 1, :], m10, axis=XY, op=ADD)
                rd(VP[:, 2, :], A8[:, :, 4:8], axis=X, op=ADD)
                rd(VP[:, 3, :], B4[:], axis=X, op=ADD)
                rd(VP[:, 4, :], G4[:], axis=X, op=ADD)
                r = p.tile([128, nsjg], F32, tag=f"r{gi}")
                nc.vector.reciprocal_approx_fast(r[:], VP[:, 4, :])
                rw.append(r)
                VT.append(VP)

            for gi, (s0_, n) in enumerate(WG):
                nsjg = n * J
                VP = VT[gi]
                r = rw[gi]
                # M rows: [t1->c3u, t2, t3, V00]; one strided GP op computes
                # [t1|t2] = [V11|V01] - [V10|V00], then c3u = t1 - t2 in
                # place, then ONE mul by the broadcast 1/D builds all of C.
                M = p.tile([128, 3, nsjg], F32, tag=f"M{gi}")
                gt(M[:, 0:2, :], VP[:, 0:3:2, :], VP[:, 1:4:2, :], op=SUB)
                gt(M[:, 2, :], VP[:, 1, :], VP[:, 3, :], op=SUB)
                gt(M[:, 0, :], M[:, 0, :], M[:, 1, :], op=SUB)
                C = p.tile([128, 4, nsjg], BF16, tag=f"C{gi}")
                rb = r[:].unsqueeze(1).broadcast_to([128, 3, nsjg])
                gt(C[:, 0:3, :], M[:], rb, op=MUL)
                gt(C[:, 3, :], VP[:, 3, :], r[:], op=MUL)
                Cpk.append(C)

            # ---- evaluate the stream tree on the 4 patterns (q-major) ----
            # m1|m4 are computed packed: P[:,0] = c3*B + c1, P[:,1] = c2*B
            # + c0, then H = P[:,0]*A + P[:,1].  4 DVE ops per layer.
            def eval_layer(l, A, Bv, cp, Hv, nf):
                P = p.tile([128, 2, 4, nf], BF16, tag=f"P{l}")
                c32 = cp[:, 0:2, :].unsqueeze(2).broadcast_to([128, 2, 4, nf])
                c10 = cp[:, 2:4, :].unsqueeze(2).broadcast_to([128, 2, 4, nf])
                bb = Bv.unsqueeze(1).broadcast_to([128, 2, 4, nf])
                nc.vector.tensor_tensor(P[:], bb, c32, op=MUL)
                nc.vector.tensor_tensor(P[:], P[:], c10, op=ADD)
                nc.vector.tensor_tensor(P[:, 0], P[:, 0], A, op=MUL)
                nc.vector.tensor_tensor(Hv, P[:, 0], P[:, 1], op=ADD)

            Hprev = None
            for l in range(L):
                nf = NS[l] * J
                H = p.tile([128, 4, nf], BF16, tag=f"H{l}")
                if l == 0:
                    A = abt[:, 0, :, :]
                    Bv = abt[:, 1, :, :]
                else:
                    A = Hprev[:, :, 0:nf]
                    Bv = Hprev[:, :, nf:2 * nf]
                g = GRP_OF_LAYER[l]
                off = (FO[l] - WG[g][0]) * J
                cp = Cpk[g][:, :, off:off + nf]
                eval_layer(l, A, Bv, cp, H[:], nf)
                Hprev = H[:]

            # ---- partial GroupSum table, broadcast across partitions ----
            Hred = p.tile([128, 4], BF16)
            with nc.allow_low_precision(reason="4-entry table; 0.4% is fine"):
                nc.vector.tensor_reduce(Hred[:], Hprev, axis=X, op=ADD)
            ps1 = psp.tile([128, 4], F32)
            nc.tensor.matmul(ps1[:], ones_m[:], Hred[:], start=True, stop=True)
            g_t = p.tile([128, 4], BF16)
            nc.scalar.copy(g_t[:], ps1[:])

            # ---- per-row blend: z2[b] = sum_q U[b,q] * table[q] ----
            W = p.tile([128, BROW, 4], BF16)
            gb = g_t[:].unsqueeze(1).broadcast_to([128, BROW, 4])
            nc.vector.tensor_tensor(W[:], U[:], gb, op=MUL)
            z2 = p.tile([128, BROW], F32)
            nc.vector.tensor_reduce(z2[:], W[:], axis=X, op=ADD)

            osb = p.tile([128, BROW, 2], F32)
            nc.vector.tensor_scalar(osb[:, :, 0], z2[:], ct[:, 0:1], None, op0=MUL)
            nc.vector.tensor_scalar(osb[:, :, 1], z2[:], ct[:, 1:2], None, op0=MUL)
            nc.sync.dma_start(out.rearrange("(p a) c -> p a c", p=128), osb[:])

    nc.compile()
    return nc


def _host_blobs(x, w0, ws, idx0, idxs):
    """Compose the stream tree and build per-core input blobs."""
    x = np.asarray(x, np.float32)
    Wl = [np.asarray(w0, np.float32)] + [np.asarray(ws[i], np.float32)
                                         for i in range(L - 1)]
    Il = [np.asarray(idx0, np.int64)] + [np.asarray(idxs[i], np.int64)
                                         for i in range(L - 1)]

    S = [None] * L
    S[L - 1] = [np.arange(K)]
    for l in range(L - 1, 0, -1):
        S[l - 1] = [Il[l][0][P] for P in S[l]] + [Il[l][1][P] for P in S[l]]

    # wall: [cores, 128, (l,s), J, 16]
    wall = np.empty((N_CORES, 128, NSTOT, J, 16), np.float32)
    for l in range(L):
        for s in range(NS[l]):
            pw = Wl[l][S[l][s]]                       # [K, 16]
            pw = pw.reshape(N_CORES, J, 128, 16)      # core, j, p, i
            wall[:, :, FO[l] + s, :, :] = pw.transpose(0, 2, 1, 3)

    # layer-0 pattern inputs: a0[core, p, s, j, q] = (q >> m0) & 1
    q = np.arange(4)
    a0 = np.empty((N_CORES, 128, NS[0], J, 4), np.float32)
    b0 = np.empty((N_CORES, 128, NS[0], J, 4), np.float32)
    for s in range(NS[0]):
        m0 = Il[0][0][S[0][s]].reshape(N_CORES, J, 128)  # core, j, p
        m1 = Il[0][1][S[0][s]].reshape(N_CORES, J, 128)
        a0[:, :, s, :, :] = ((q[None, None, None, :] >> m0.transpose(0, 2, 1)[..., None]) & 1)
        b0[:, :, s, :, :] = ((q[None, None, None, :] >> m1.transpose(0, 2, 1)[..., None]) & 1)
    # q-major: [c, p, q, s*j]
    a0 = a0.transpose(0, 1, 4, 2, 3).reshape(N_CORES, 128, 4, NS[0] * J)
    b0 = b0.transpose(0, 1, 4, 2, 3).reshape(N_CORES, 128, 4, NS[0] * J)

    xin = np.ascontiguousarray(x.reshape(128, BROW, 2))
    in_maps = []
    for ci in range(N_CORES):
        cls = np.array([1.0, 0.0] if ci < N_CORES // 2 else [0.0, 1.0], np.float32)
        xcls = np.concatenate(
            [xin, np.tile(cls, (128, 1))[:, None, :]], axis=1)
        ab0 = np.stack([a0[ci], b0[ci]], axis=1).astype(ml_dtypes.bfloat16)
        m = {
            "ab0in": np.ascontiguousarray(ab0),
            "xclsin": np.ascontiguousarray(xcls),
        }
        for gi, (gs, gn) in enumerate(WG):
            m[f"wall{gi}"] = np.ascontiguousarray(
                wall[ci, :, gs:gs + gn, :, :].reshape(128, -1)).astype(
                    ml_dtypes.float8_e3m4)
        in_maps.append(m)
    return in_maps


def run(inputs, trace=False, trace_kwargs=None):
    global _compiled
    if _compiled is None:
        _compiled = _build_program()
    nc = _compiled
    in_maps = _host_blobs(inputs["x"], inputs["w0"], inputs["ws"],
                          inputs["idx0"], inputs["idxs"])
    res = run_bass_kernel_spmd(nc, in_maps, core_ids=list(range(N_CORES)),
                               trace=trace, **(trace_kwargs or {}))
    total = np.zeros((B, 2), np.float32)
    for ci in range(N_CORES):
        total += res.results[ci]["out"]
    return total, res


def kernel(x, w0, ws, idx0, idxs):
    out, _ = run({"x": x, "w0": w0, "ws": ws, "idx0": idx0, "idxs": idxs})
    return out
